# revision 18
# baseline (speedup 1.0000x reference)
"""GCN layer (4-relation message passing) on 8 Trainium2 NeuronCores.

out = sum_r (A_r @ inp) @ W_r + sum_r b_r,  A_r in COO form (dst, src, val).

Sharding: edges sharded by dst range; core c owns dst in [c*12500, (c+1)*12500).

Design (~600us HW vs 709us for the shipped-one-hot baseline): the selection
matrix is built ON DEVICE, so per edge the host ships only the 64-col bf16
message row (pure placement of inp[src]) plus 2+2 bytes of metadata
(wloc, val as bf16), cutting slab DMA from ~233MB to ~134MB per core.

Per (32-node dst window w, relation r) cell, per 128-edge block b:
  sel_b[p, j] = val_p * (j == wloc_p)   built by two wide DVE passes in
  j-major layout (inner dense step keeps the DVE 2x mode; a stride-0
  innermost broadcast operand would demote to 1x):
                is_equal(iota_jmajor, wloc_bcast) then mult by val_bcast
  PE: aggT_wr [64feat, 32nodes] += MSG_b^T @ sel_b  (PSUM, col-tiled:
      rel 0/1 -> PSUM partitions 0:64 / 64:128 of one [128,32] tile)
Stage 2 per window (bf16 - fp32 matmuls emit two PE instrs): po [32, 64]
  = aggT_01-stack @ [W0;W1] + aggT_23-stack @ [W2;W3].
Bias is summed on device ONCE (ones.T @ bias matmul), row-replicated to
128 partitions, and added to the output staging tile by chunked wide DVE
adds instead of a per-window matmul.
All DMA-group loads + sel builds are emitted upfront (Tile priority =
emission order; lazy emission caused 300us of PE stalls). wloc/val slabs
are SBUF-resident as four per-chunk tiles (whole-tile dep granularity:
one tile would gate the first sel build on the whole slab landing).
Host: unscramble windows, no arithmetic beyond dtype casts.
"""

import math
from contextlib import ExitStack

import numpy as np

import concourse.bass as bass
import concourse.tile as tile
from concourse import bacc, mybir
from concourse.bass_utils import run_bass_kernel_spmd

# problem constants
N_NODES = 100000
N_REL = 4
N_EDGES = 1600000
IN_SIZE = 64
OUT_SIZE = 64

N_CORES = 8
NPC = N_NODES // N_CORES  # nodes (dst) per core
P = 128                   # partitions / edges per block
W = 32                    # dst-window width (nodes per psum column range)
WPP = P // W              # windows packed per partition block of outsb
GW = 96                   # blocks per DMA group (12KB/partition per msg DMA)
NMC = 4                   # metadata chunk tiles

F32 = mybir.dt.float32
BF16 = mybir.dt.bfloat16


def _bf16():
    import ml_dtypes
    return ml_dtypes.bfloat16


def _host_prep(inp, src, dst, edge_val):
    """Bucket/pad edges per (core, window, rel); build msg/wloc/val slabs."""
    n_win = math.ceil(NPC / W)
    ncell = n_win * N_REL
    srcf = src.reshape(-1).astype(np.int64)
    dstf = dst.reshape(-1).astype(np.int64)
    valf = edge_val.reshape(-1).astype(np.float32)
    rel = np.repeat(np.arange(N_REL, dtype=np.int64), src.shape[1])

    core = dstf // NPC
    dloc = dstf % NPC
    win = dloc // W
    wloc = dloc % W
    cell = win * N_REL + rel
    key = core * ncell + cell

    counts = np.bincount(key, minlength=N_CORES * ncell).reshape(
        N_CORES, ncell)
    B = np.maximum((counts.max(axis=0) + P - 1) // P, 1).astype(np.int64)
    # pad total block count to a multiple of GW so every DMA group is full
    pad = (-int(B.sum())) % GW
    B[-1] += pad
    starts = np.zeros(ncell + 1, dtype=np.int64)
    np.cumsum(B, out=starts[1:])
    T = int(starts[-1])

    bf16 = _bf16()
    msg_all = np.zeros((N_CORES, P, T, IN_SIZE), dtype=bf16)
    wloc_all = np.full((N_CORES, P, T), -1.0, dtype=bf16)
    val_all = np.zeros((N_CORES, P, T), dtype=bf16)

    order = np.argsort(key, kind="stable")
    grp_start = np.zeros(N_CORES * ncell, dtype=np.int64)
    np.cumsum(counts.reshape(-1)[:-1], out=grp_start[1:])
    j = np.arange(len(order), dtype=np.int64) - grp_start[key[order]]
    t_col = starts[cell[order]] + (j // P)
    p_row = j % P
    c_ord = core[order]
    msg_all[c_ord, p_row, t_col, :] = inp[srcf[order]].astype(bf16)
    wloc_all[c_ord, p_row, t_col] = wloc[order].astype(bf16)
    val_all[c_ord, p_row, t_col] = valf[order].astype(bf16)

    return n_win, starts, T, msg_all, wloc_all, val_all


_PROG_CACHE = {}


def _build_program(n_win, starts, T):
    key = (W, GW, n_win, tuple(int(s) for s in starts), T)
    if key in _PROG_CACHE:
        return _PROG_CACHE[key]

    nc = bacc.Bacc("TRN2", target_bir_lowering=False, debug=False,
                   num_devices=N_CORES)
    wst = nc.dram_tensor("wst", [2 * IN_SIZE, 2 * OUT_SIZE], BF16,
                         kind="ExternalInput").ap()
    biasd = nc.dram_tensor("biasd", [N_REL, OUT_SIZE], BF16,
                           kind="ExternalInput").ap()
    onesd = nc.dram_tensor("onesd", [N_REL, W], BF16,
                           kind="ExternalInput").ap()
    iotad = nc.dram_tensor("iotad", [P, GW * W], BF16,
                           kind="ExternalInput").ap()
    emsg = nc.dram_tensor("emsg", [P, T * IN_SIZE], BF16,
                          kind="ExternalInput").ap()
    ewlc = nc.dram_tensor("ewlc", [P, T], BF16, kind="ExternalInput").ap()
    eval_ = nc.dram_tensor("eval", [P, T], BF16, kind="ExternalInput").ap()
    n_wcol = (n_win + WPP - 1) // WPP
    out = nc.dram_tensor("out", [P, n_wcol * OUT_SIZE], F32,
                         kind="ExternalOutput").ap()

    ngroups = (T + GW - 1) // GW
    # metadata chunk size in groups / slots (chunks aligned to GW)
    cg = (ngroups + NMC - 1) // NMC
    mc = cg * GW

    with tile.TileContext(nc) as tc, ExitStack() as ctx:
        p_c = ctx.enter_context(tc.tile_pool(name="p_c", bufs=1))
        p_msg = ctx.enter_context(tc.tile_pool(name="p_msg", bufs=4))
        p_sel = ctx.enter_context(tc.tile_pool(name="p_sel", bufs=4))
        p_agg = ctx.enter_context(tc.tile_pool(name="p_agg", bufs=4))
        ps_agg = ctx.enter_context(tc.tile_pool(name="ps_agg", bufs=6,
                                                space="PSUM"))
        ps_out = ctx.enter_context(tc.tile_pool(name="ps_out", bufs=2,
                                                space="PSUM"))

        iot = p_c.tile([P, GW * W], BF16)
        nc.sync.dma_start(iot[:], iotad[:])
        # metadata resident in SBUF as NMC chunk tiles (per-tile deps let
        # early sel builds start before later chunks land)
        wlc_t, vlc_t = [], []
        for c in range(NMC):
            c0, c1 = c * mc, min((c + 1) * mc, T)
            wl_c = p_c.tile([P, mc], BF16, tag=f"wlc{c}")
            vl_c = p_c.tile([P, mc], BF16, tag=f"vlc{c}")
            wlc_t.append(wl_c)
            vlc_t.append(vl_c)
            if c > 0:
                continue
            nc.sync.dma_start(wl_c[:, :c1 - c0], ewlc[:, c0:c1])
            nc.sync.dma_start(vl_c[:, :c1 - c0], eval_[:, c0:c1])

        groups = {}

        def get_group(g):
            if g not in groups:
                g0, g1 = g * GW, min((g + 1) * GW, T)
                nb = g1 - g0
                mt = p_msg.tile([P, GW * IN_SIZE], BF16, tag="msg")
                nc.scalar.dma_start(mt[:, :nb * IN_SIZE],
                                    emsg[:, g0 * IN_SIZE:g1 * IN_SIZE])
                # selection built j-major: st[p, j*nb + v]; DVE broadcast
                # operands keep an inner dense step (2x mode)
                st = p_sel.tile([P, GW * W], BF16, tag="sel")
                s3 = st[:, :nb * W].rearrange("p (j v) -> p j v", j=W)
                i3 = iot[:, :nb * W].rearrange("p (j v) -> p j v", j=W)
                ci = g // cg
                o0 = g0 - ci * mc
                bw = wlc_t[ci][:, o0:o0 + nb].unsqueeze(1).broadcast_to(
                    [P, W, nb])
                bv = vlc_t[ci][:, o0:o0 + nb].unsqueeze(1).broadcast_to(
                    [P, W, nb])
                nc.vector.tensor_tensor(s3, i3, bw,
                                        op=mybir.AluOpType.is_equal)
                nc.vector.tensor_tensor(s3, s3, bv, op=mybir.AluOpType.mult)
                groups[g] = (mt, st, nb)
            return groups[g]

        def cell_ap(b):
            mt, st, nb = get_group(b // GW)
            o = b % GW
            return (mt[:, o * IN_SIZE:(o + 1) * IN_SIZE],
                    st[:, :nb * W].rearrange("p (j v) -> p j v", j=W)[:, :, o])

        # prime the pipeline before emitting the remaining constant loads
        for g in range(min(4, ngroups)):
            get_group(g)

        wt = p_c.tile([2 * IN_SIZE, 2 * OUT_SIZE], BF16)
        nc.sync.dma_start(wt[:], wst[:])
        bt = p_c.tile([N_REL, OUT_SIZE], BF16)
        nc.sync.dma_start(bt[:], biasd[:])
        ot = p_c.tile([N_REL, W], BF16)
        nc.sync.dma_start(ot[:], onesd[:])
        for c in range(1, NMC):
            c0, c1 = c * mc, min((c + 1) * mc, T)
            nc.sync.dma_start(wlc_t[c][:, :c1 - c0], ewlc[:, c0:c1])
            nc.sync.dma_start(vlc_t[c][:, :c1 - c0], eval_[:, c0:c1])
        outsb = p_c.tile([P, n_wcol * OUT_SIZE], F32)

        # bias summed over relations once: pb = ones.T @ bias  [W, OUT]
        po = ps_out.tile([W, OUT_SIZE], F32)
        nc.tensor.matmul(out=po[:], lhsT=ot[:], rhs=bt[:],
                         start=True, stop=True)
        bb32 = p_c.tile([W, OUT_SIZE], F32)
        nc.vector.tensor_copy(bb32[:], po[:])
        # replicate to all 128 partitions (partition-offset copies)
        bbt = p_c.tile([P, OUT_SIZE], F32)
        k0 = 0
        while k0 < P:
            sz = min(W, P - k0)
            nc.vector.tensor_copy(bbt[k0:k0 + sz, :], bb32[:sz, :])
            k0 += sz

        for g in range(4, ngroups):
            get_group(g)

        for w in range(n_win):
            aggs = []
            for half in range(2):
                ps = ps_agg.tile([P, W], F32)
                for rr in range(2):
                    c2 = w * N_REL + 2 * half + rr
                    b0, b1 = int(starts[c2]), int(starts[c2 + 1])
                    for k, b in enumerate(range(b0, b1)):
                        m_ap, s_ap = cell_ap(b)
                        nc.tensor.matmul(
                            out=ps[rr * IN_SIZE:(rr + 1) * IN_SIZE, :],
                            lhsT=m_ap, rhs=s_ap,
                            tile_position=(0, rr * IN_SIZE),
                            start=(k == 0), stop=(b == b1 - 1))
                agg = p_agg.tile([P, W], BF16, tag="agg")
                nc.scalar.copy(agg[:], ps[:])
                aggs.append(agg)
            po = ps_out.tile([W, OUT_SIZE], F32)
            nc.tensor.matmul(out=po[:], lhsT=aggs[0][:],
                             rhs=wt[:, :OUT_SIZE], start=True, stop=False)
            nc.tensor.matmul(out=po[:], lhsT=aggs[1][:],
                             rhs=wt[:, OUT_SIZE:], start=False, stop=True)
            nc.scalar.copy(
                outsb[(w % WPP) * W:(w % WPP) * W + W,
                      (w // WPP) * OUT_SIZE:(w // WPP + 1) * OUT_SIZE],
                po[:])
        noc = 4
        oc = (n_wcol + noc - 1) // noc
        for c in range(noc):
            k0, k1 = c * oc, min((c + 1) * oc, n_wcol)
            c0, c1 = k0 * OUT_SIZE, k1 * OUT_SIZE
            o3 = outsb[:, c0:c1].rearrange("p (k o) -> p k o", o=OUT_SIZE)
            bb = bbt[:].unsqueeze(1).broadcast_to([P, k1 - k0, OUT_SIZE])
            nc.vector.tensor_tensor(o3, o3, bb, op=mybir.AluOpType.add)
            nc.sync.dma_start(out[:, c0:c1], outsb[:, c0:c1])

    nc.compile()
    _PROG_CACHE[key] = nc
    return nc


def _prepare(inp, src, dst, edge_val, weights, bias):
    inp = np.asarray(inp, dtype=np.float32)
    src = np.asarray(src)
    dst = np.asarray(dst)
    edge_val = np.asarray(edge_val, dtype=np.float32)
    weights = np.asarray(weights, dtype=np.float32)
    bias = np.asarray(bias, dtype=np.float32)

    n_win, starts, T, msg_all, wloc_all, val_all = _host_prep(
        inp, src, dst, edge_val)
    nc = _build_program(n_win, starts, T)

    bf16 = _bf16()
    wst = np.zeros((2 * IN_SIZE, 2 * OUT_SIZE), dtype=bf16)
    wst[:IN_SIZE, :OUT_SIZE] = weights[0].astype(bf16)
    wst[IN_SIZE:, :OUT_SIZE] = weights[1].astype(bf16)
    wst[:IN_SIZE, OUT_SIZE:] = weights[2].astype(bf16)
    wst[IN_SIZE:, OUT_SIZE:] = weights[3].astype(bf16)
    ones = np.ones((N_REL, W), dtype=bf16)
    # j-major iota: value at free position j*GW + v is j
    iota = np.broadcast_to(
        np.repeat(np.arange(W, dtype=np.float32), GW).astype(bf16),
        (P, GW * W)).copy()

    in_maps = []
    for c in range(N_CORES):
        in_maps.append({
            "wst": wst,
            "biasd": bias.astype(bf16),
            "onesd": ones,
            "iotad": iota,
            "emsg": msg_all[c].reshape(P, T * IN_SIZE),
            "ewlc": wloc_all[c],
            "eval": val_all[c],
        })
    return nc, in_maps, n_win


def _finish(res, n_win):
    n_wcol = (n_win + WPP - 1) // WPP
    parts = []
    for c in range(N_CORES):
        arr = res.results[c]["out"].reshape(P, n_wcol, OUT_SIZE)
        arr = arr[:WPP * W].reshape(WPP, W, n_wcol, OUT_SIZE)
        nodes = arr.transpose(2, 0, 1, 3).reshape(n_wcol * WPP * W, OUT_SIZE)
        parts.append(nodes[:NPC])
    return np.concatenate(parts, axis=0).astype(np.float32)


def kernel(inp, src, dst, edge_val, weights, bias):
    nc, in_maps, n_win = _prepare(inp, src, dst, edge_val, weights, bias)
    res = run_bass_kernel_spmd(nc, in_maps, list(range(N_CORES)))
    return _finish(res, n_win)


# revision 19
# speedup vs baseline: 1.0099x; 1.0099x over previous
"""GCN layer (4-relation message passing) on 8 Trainium2 NeuronCores.

out = sum_r (A_r @ inp) @ W_r + sum_r b_r,  A_r in COO form (dst, src, val).

Sharding: edges sharded by dst range; core c owns dst in [c*12500, (c+1)*12500).

Design (~600us HW vs 709us for the shipped-one-hot baseline): the selection
matrix is built ON DEVICE, so per edge the host ships only the 64-col bf16
message row (pure placement of inp[src]) plus 2+2 bytes of metadata
(wloc, val as bf16), cutting slab DMA from ~233MB to ~134MB per core.

Per (32-node dst window w, relation r) cell, per 128-edge block b:
  sel_b[p, j] = val_p * (j == wloc_p)   built by two wide DVE passes in
  j-major layout (inner dense step keeps the DVE 2x mode; a stride-0
  innermost broadcast operand would demote to 1x):
                is_equal(iota_jmajor, wloc_bcast) then mult by val_bcast
  PE: aggT_wr [64feat, 32nodes] += MSG_b^T @ sel_b  (PSUM, col-tiled:
      rel 0/1 -> PSUM partitions 0:64 / 64:128 of one [128,32] tile)
Stage 2 per window (bf16 - fp32 matmuls emit two PE instrs): po [32, 64]
  = aggT_01-stack @ [W0;W1] + aggT_23-stack @ [W2;W3].
Bias is summed on device ONCE (ones.T @ bias matmul), row-replicated to
128 partitions, and added to the output staging tile by chunked wide DVE
adds instead of a per-window matmul.
All DMA-group loads + sel builds are emitted upfront (Tile priority =
emission order; lazy emission caused 300us of PE stalls). wloc/val slabs
are SBUF-resident as four per-chunk tiles (whole-tile dep granularity:
one tile would gate the first sel build on the whole slab landing).
Host: unscramble windows, no arithmetic beyond dtype casts.
"""

import math
from contextlib import ExitStack

import numpy as np

import concourse.bass as bass
import concourse.tile as tile
from concourse import bacc, mybir
from concourse.bass_utils import run_bass_kernel_spmd

# problem constants
N_NODES = 100000
N_REL = 4
N_EDGES = 1600000
IN_SIZE = 64
OUT_SIZE = 64

N_CORES = 8
NPC = N_NODES // N_CORES  # nodes (dst) per core
P = 128                   # partitions / edges per block
W = 32                    # dst-window width (nodes per psum column range)
WPP = P // W              # windows packed per partition block of outsb
GW = 96                   # blocks per DMA group (12KB/partition per msg DMA)
NMC = 4                   # metadata chunk tiles

F32 = mybir.dt.float32
BF16 = mybir.dt.bfloat16


def _bf16():
    import ml_dtypes
    return ml_dtypes.bfloat16


def _host_prep(inp, src, dst, edge_val):
    """Bucket/pad edges per (core, window, rel); build msg/wloc/val slabs."""
    n_win = math.ceil(NPC / W)
    ncell = n_win * N_REL
    srcf = src.reshape(-1).astype(np.int64)
    dstf = dst.reshape(-1).astype(np.int64)
    valf = edge_val.reshape(-1).astype(np.float32)
    rel = np.repeat(np.arange(N_REL, dtype=np.int64), src.shape[1])

    core = dstf // NPC
    dloc = dstf % NPC
    win = dloc // W
    wloc = dloc % W
    cell = win * N_REL + rel
    key = core * ncell + cell

    counts = np.bincount(key, minlength=N_CORES * ncell).reshape(
        N_CORES, ncell)
    B = np.maximum((counts.max(axis=0) + P - 1) // P, 1).astype(np.int64)
    # pad total block count to a multiple of GW so every DMA group is full
    pad = (-int(B.sum())) % GW
    B[-1] += pad
    starts = np.zeros(ncell + 1, dtype=np.int64)
    np.cumsum(B, out=starts[1:])
    T = int(starts[-1])

    bf16 = _bf16()
    msg_all = np.zeros((N_CORES, P, T, IN_SIZE), dtype=bf16)
    wloc_all = np.full((N_CORES, P, T), -1.0, dtype=bf16)
    val_all = np.zeros((N_CORES, P, T), dtype=bf16)

    order = np.argsort(key, kind="stable")
    grp_start = np.zeros(N_CORES * ncell, dtype=np.int64)
    np.cumsum(counts.reshape(-1)[:-1], out=grp_start[1:])
    j = np.arange(len(order), dtype=np.int64) - grp_start[key[order]]
    t_col = starts[cell[order]] + (j // P)
    p_row = j % P
    c_ord = core[order]
    msg_all[c_ord, p_row, t_col, :] = inp[srcf[order]].astype(bf16)
    wloc_all[c_ord, p_row, t_col] = wloc[order].astype(bf16)
    val_all[c_ord, p_row, t_col] = valf[order].astype(bf16)

    return n_win, starts, T, msg_all, wloc_all, val_all


_PROG_CACHE = {}


def _build_program(n_win, starts, T):
    key = (W, GW, n_win, tuple(int(s) for s in starts), T)
    if key in _PROG_CACHE:
        return _PROG_CACHE[key]

    nc = bacc.Bacc("TRN2", target_bir_lowering=False, debug=False,
                   num_devices=N_CORES)
    wst = nc.dram_tensor("wst", [2 * IN_SIZE, 2 * OUT_SIZE], BF16,
                         kind="ExternalInput").ap()
    biasd = nc.dram_tensor("biasd", [N_REL, OUT_SIZE], BF16,
                           kind="ExternalInput").ap()
    onesd = nc.dram_tensor("onesd", [N_REL, W], BF16,
                           kind="ExternalInput").ap()
    iotad = nc.dram_tensor("iotad", [P, GW * W], BF16,
                           kind="ExternalInput").ap()
    emsg = nc.dram_tensor("emsg", [P, T * IN_SIZE], BF16,
                          kind="ExternalInput").ap()
    ewlc = nc.dram_tensor("ewlc", [P, T], BF16, kind="ExternalInput").ap()
    eval_ = nc.dram_tensor("eval", [P, T], BF16, kind="ExternalInput").ap()
    n_wcol = (n_win + WPP - 1) // WPP
    out = nc.dram_tensor("out", [P, n_wcol * OUT_SIZE], F32,
                         kind="ExternalOutput").ap()

    ngroups = (T + GW - 1) // GW
    # metadata chunk size in groups / slots (chunks aligned to GW)
    cg = (ngroups + NMC - 1) // NMC
    mc = cg * GW

    with tile.TileContext(nc) as tc, ExitStack() as ctx:
        p_c = ctx.enter_context(tc.tile_pool(name="p_c", bufs=1))
        p_msg = ctx.enter_context(tc.tile_pool(name="p_msg", bufs=5))
        p_sel = ctx.enter_context(tc.tile_pool(name="p_sel", bufs=5))
        p_agg = ctx.enter_context(tc.tile_pool(name="p_agg", bufs=6))
        ps_agg = ctx.enter_context(tc.tile_pool(name="ps_agg", bufs=6,
                                                space="PSUM"))
        ps_out = ctx.enter_context(tc.tile_pool(name="ps_out", bufs=2,
                                                space="PSUM"))

        iot = p_c.tile([P, GW * W], BF16)
        nc.sync.dma_start(iot[:], iotad[:])
        # metadata resident in SBUF as NMC chunk tiles (per-tile deps let
        # early sel builds start before later chunks land)
        wlc_t, vlc_t = [], []
        for c in range(NMC):
            c0, c1 = c * mc, min((c + 1) * mc, T)
            wl_c = p_c.tile([P, mc], BF16, tag=f"wlc{c}")
            vl_c = p_c.tile([P, mc], BF16, tag=f"vlc{c}")
            wlc_t.append(wl_c)
            vlc_t.append(vl_c)
            if c > 0:
                continue
            nc.sync.dma_start(wl_c[:, :c1 - c0], ewlc[:, c0:c1])
            nc.sync.dma_start(vl_c[:, :c1 - c0], eval_[:, c0:c1])

        groups = {}

        def get_group(g):
            if g not in groups:
                g0, g1 = g * GW, min((g + 1) * GW, T)
                nb = g1 - g0
                mt = p_msg.tile([P, GW * IN_SIZE], BF16, tag="msg")
                nc.scalar.dma_start(mt[:, :nb * IN_SIZE],
                                    emsg[:, g0 * IN_SIZE:g1 * IN_SIZE])
                # selection built j-major: st[p, j*nb + v]; DVE broadcast
                # operands keep an inner dense step (2x mode)
                st = p_sel.tile([P, GW * W], BF16, tag="sel")
                s3 = st[:, :nb * W].rearrange("p (j v) -> p j v", j=W)
                i3 = iot[:, :nb * W].rearrange("p (j v) -> p j v", j=W)
                ci = g // cg
                o0 = g0 - ci * mc
                bw = wlc_t[ci][:, o0:o0 + nb].unsqueeze(1).broadcast_to(
                    [P, W, nb])
                bv = vlc_t[ci][:, o0:o0 + nb].unsqueeze(1).broadcast_to(
                    [P, W, nb])
                nc.vector.tensor_tensor(s3, i3, bw,
                                        op=mybir.AluOpType.is_equal)
                nc.vector.tensor_tensor(s3, s3, bv, op=mybir.AluOpType.mult)
                groups[g] = (mt, st, nb)
            return groups[g]

        def cell_ap(b):
            mt, st, nb = get_group(b // GW)
            o = b % GW
            return (mt[:, o * IN_SIZE:(o + 1) * IN_SIZE],
                    st[:, :nb * W].rearrange("p (j v) -> p j v", j=W)[:, :, o])

        # prime the pipeline before emitting the remaining constant loads
        for g in range(min(4, ngroups)):
            get_group(g)

        wt = p_c.tile([2 * IN_SIZE, 2 * OUT_SIZE], BF16)
        nc.sync.dma_start(wt[:], wst[:])
        bt = p_c.tile([N_REL, OUT_SIZE], BF16)
        nc.sync.dma_start(bt[:], biasd[:])
        ot = p_c.tile([N_REL, W], BF16)
        nc.sync.dma_start(ot[:], onesd[:])
        for c in range(1, NMC):
            c0, c1 = c * mc, min((c + 1) * mc, T)
            nc.sync.dma_start(wlc_t[c][:, :c1 - c0], ewlc[:, c0:c1])
            nc.sync.dma_start(vlc_t[c][:, :c1 - c0], eval_[:, c0:c1])
        outsb = p_c.tile([P, n_wcol * OUT_SIZE], F32)

        # bias summed over relations once: pb = ones.T @ bias  [W, OUT]
        po = ps_out.tile([W, OUT_SIZE], F32)
        nc.tensor.matmul(out=po[:], lhsT=ot[:], rhs=bt[:],
                         start=True, stop=True)
        bb32 = p_c.tile([W, OUT_SIZE], F32)
        nc.vector.tensor_copy(bb32[:], po[:])
        # replicate to all 128 partitions (partition-offset copies)
        bbt = p_c.tile([P, OUT_SIZE], F32)
        k0 = 0
        while k0 < P:
            sz = min(W, P - k0)
            nc.vector.tensor_copy(bbt[k0:k0 + sz, :], bb32[:sz, :])
            k0 += sz

        for g in range(4, ngroups):
            get_group(g)

        for w in range(n_win):
            aggs = []
            for half in range(2):
                ps = ps_agg.tile([P, W], F32)
                for rr in range(2):
                    c2 = w * N_REL + 2 * half + rr
                    b0, b1 = int(starts[c2]), int(starts[c2 + 1])
                    for k, b in enumerate(range(b0, b1)):
                        m_ap, s_ap = cell_ap(b)
                        nc.tensor.matmul(
                            out=ps[rr * IN_SIZE:(rr + 1) * IN_SIZE, :],
                            lhsT=m_ap, rhs=s_ap,
                            tile_position=(0, rr * IN_SIZE),
                            start=(k == 0), stop=(b == b1 - 1))
                agg = p_agg.tile([P, W], BF16, tag="agg")
                nc.scalar.copy(agg[:], ps[:])
                aggs.append(agg)
            po = ps_out.tile([W, OUT_SIZE], F32)
            nc.tensor.matmul(out=po[:], lhsT=aggs[0][:],
                             rhs=wt[:, :OUT_SIZE], start=True, stop=False)
            nc.tensor.matmul(out=po[:], lhsT=aggs[1][:],
                             rhs=wt[:, OUT_SIZE:], start=False, stop=True)
            nc.scalar.copy(
                outsb[(w % WPP) * W:(w % WPP) * W + W,
                      (w // WPP) * OUT_SIZE:(w // WPP + 1) * OUT_SIZE],
                po[:])
        noc = 4
        oc = (n_wcol + noc - 1) // noc
        for c in range(noc):
            k0, k1 = c * oc, min((c + 1) * oc, n_wcol)
            c0, c1 = k0 * OUT_SIZE, k1 * OUT_SIZE
            o3 = outsb[:, c0:c1].rearrange("p (k o) -> p k o", o=OUT_SIZE)
            bb = bbt[:].unsqueeze(1).broadcast_to([P, k1 - k0, OUT_SIZE])
            nc.vector.tensor_tensor(o3, o3, bb, op=mybir.AluOpType.add)
            nc.sync.dma_start(out[:, c0:c1], outsb[:, c0:c1])

    nc.compile()
    _PROG_CACHE[key] = nc
    return nc


def _prepare(inp, src, dst, edge_val, weights, bias):
    inp = np.asarray(inp, dtype=np.float32)
    src = np.asarray(src)
    dst = np.asarray(dst)
    edge_val = np.asarray(edge_val, dtype=np.float32)
    weights = np.asarray(weights, dtype=np.float32)
    bias = np.asarray(bias, dtype=np.float32)

    n_win, starts, T, msg_all, wloc_all, val_all = _host_prep(
        inp, src, dst, edge_val)
    nc = _build_program(n_win, starts, T)

    bf16 = _bf16()
    wst = np.zeros((2 * IN_SIZE, 2 * OUT_SIZE), dtype=bf16)
    wst[:IN_SIZE, :OUT_SIZE] = weights[0].astype(bf16)
    wst[IN_SIZE:, :OUT_SIZE] = weights[1].astype(bf16)
    wst[:IN_SIZE, OUT_SIZE:] = weights[2].astype(bf16)
    wst[IN_SIZE:, OUT_SIZE:] = weights[3].astype(bf16)
    ones = np.ones((N_REL, W), dtype=bf16)
    # j-major iota: value at free position j*GW + v is j
    iota = np.broadcast_to(
        np.repeat(np.arange(W, dtype=np.float32), GW).astype(bf16),
        (P, GW * W)).copy()

    in_maps = []
    for c in range(N_CORES):
        in_maps.append({
            "wst": wst,
            "biasd": bias.astype(bf16),
            "onesd": ones,
            "iotad": iota,
            "emsg": msg_all[c].reshape(P, T * IN_SIZE),
            "ewlc": wloc_all[c],
            "eval": val_all[c],
        })
    return nc, in_maps, n_win


def _finish(res, n_win):
    n_wcol = (n_win + WPP - 1) // WPP
    parts = []
    for c in range(N_CORES):
        arr = res.results[c]["out"].reshape(P, n_wcol, OUT_SIZE)
        arr = arr[:WPP * W].reshape(WPP, W, n_wcol, OUT_SIZE)
        nodes = arr.transpose(2, 0, 1, 3).reshape(n_wcol * WPP * W, OUT_SIZE)
        parts.append(nodes[:NPC])
    return np.concatenate(parts, axis=0).astype(np.float32)


def kernel(inp, src, dst, edge_val, weights, bias):
    nc, in_maps, n_win = _prepare(inp, src, dst, edge_val, weights, bias)
    res = run_bass_kernel_spmd(nc, in_maps, list(range(N_CORES)))
    return _finish(res, n_win)


# revision 20
# speedup vs baseline: 1.0822x; 1.0716x over previous
"""GCN layer (4-relation message passing) on 8 Trainium2 NeuronCores.

out = sum_r (A_r @ inp) @ W_r + sum_r b_r,  A_r in COO form (dst, src, val).

Sharding: edges sharded by dst range; core c owns dst in [c*12500, (c+1)*12500).

Design (~600us HW vs 709us for the shipped-one-hot baseline): the selection
matrix is built ON DEVICE, so per edge the host ships only the 64-col bf16
message row (pure placement of inp[src]) plus 2+2 bytes of metadata
(wloc, val as bf16), cutting slab DMA from ~233MB to ~134MB per core.

Per (32-node dst window w, relation r) cell, per 128-edge block b:
  sel_b[p, j] = val_p * (j == wloc_p)   built by two wide DVE passes in
  j-major layout (inner dense step keeps the DVE 2x mode; a stride-0
  innermost broadcast operand would demote to 1x):
                is_equal(iota_jmajor, wloc_bcast) then mult by val_bcast
  PE: aggT_wr [64feat, 32nodes] += MSG_b^T @ sel_b  (PSUM, col-tiled:
      rel 0/1 -> PSUM partitions 0:64 / 64:128 of one [128,32] tile)
Stage 2 per window (bf16 - fp32 matmuls emit two PE instrs): po [32, 64]
  = aggT_01-stack @ [W0;W1] + aggT_23-stack @ [W2;W3].
Bias is summed on device ONCE (ones.T @ bias matmul), row-replicated to
128 partitions, and added to the output staging tile by chunked wide DVE
adds instead of a per-window matmul.
All DMA-group loads + sel builds are emitted upfront (Tile priority =
emission order; lazy emission caused 300us of PE stalls). wloc/val slabs
are SBUF-resident as four per-chunk tiles (whole-tile dep granularity:
one tile would gate the first sel build on the whole slab landing).
Host: unscramble windows, no arithmetic beyond dtype casts.
"""

import math
from contextlib import ExitStack

import numpy as np

import concourse.bass as bass
import concourse.tile as tile
from concourse import bacc, mybir
from concourse.bass_utils import run_bass_kernel_spmd

# problem constants
N_NODES = 100000
N_REL = 4
N_EDGES = 1600000
IN_SIZE = 64
OUT_SIZE = 64

N_CORES = 8
NPC = N_NODES // N_CORES  # nodes (dst) per core
P = 128                   # partitions / edges per block
W = 32                    # dst-window width (nodes per psum column range)
WPP = P // W              # windows packed per partition block of outsb
GW = 96                   # blocks per DMA group (12KB/partition per msg DMA)
NMC = 4                   # metadata chunk tiles

F32 = mybir.dt.float32
BF16 = mybir.dt.bfloat16


def _bf16():
    import ml_dtypes
    return ml_dtypes.bfloat16


def _host_prep(inp, src, dst, edge_val):
    """Bucket/pad edges per (core, window, rel); build msg/wloc/val slabs."""
    n_win = math.ceil(NPC / W)
    ncell = n_win * N_REL
    srcf = src.reshape(-1).astype(np.int64)
    dstf = dst.reshape(-1).astype(np.int64)
    valf = edge_val.reshape(-1).astype(np.float32)
    rel = np.repeat(np.arange(N_REL, dtype=np.int64), src.shape[1])

    core = dstf // NPC
    dloc = dstf % NPC
    win = dloc // W
    wloc = dloc % W
    cell = win * N_REL + rel
    key = core * ncell + cell

    counts = np.bincount(key, minlength=N_CORES * ncell).reshape(
        N_CORES, ncell)
    B = np.maximum((counts.max(axis=0) + P - 1) // P, 1).astype(np.int64)
    # pad total block count to a multiple of GW so every DMA group is full
    pad = (-int(B.sum())) % GW
    B[-1] += pad
    starts = np.zeros(ncell + 1, dtype=np.int64)
    np.cumsum(B, out=starts[1:])
    T = int(starts[-1])

    bf16 = _bf16()
    msg_all = np.zeros((N_CORES, P, T, IN_SIZE), dtype=bf16)
    wloc_all = np.full((N_CORES, P, T), -1.0, dtype=bf16)
    val_all = np.zeros((N_CORES, P, T), dtype=bf16)

    order = np.argsort(key, kind="stable")
    grp_start = np.zeros(N_CORES * ncell, dtype=np.int64)
    np.cumsum(counts.reshape(-1)[:-1], out=grp_start[1:])
    j = np.arange(len(order), dtype=np.int64) - grp_start[key[order]]
    t_col = starts[cell[order]] + (j // P)
    p_row = j % P
    c_ord = core[order]
    msg_all[c_ord, p_row, t_col, :] = inp[srcf[order]].astype(bf16)
    wloc_all[c_ord, p_row, t_col] = wloc[order].astype(bf16)
    val_all[c_ord, p_row, t_col] = valf[order].astype(bf16)

    return n_win, starts, T, msg_all, wloc_all, val_all


_PROG_CACHE = {}


def _build_program(n_win, starts, T):
    key = (W, GW, n_win, tuple(int(s) for s in starts), T)
    if key in _PROG_CACHE:
        return _PROG_CACHE[key]

    nc = bacc.Bacc("TRN2", target_bir_lowering=False, debug=False,
                   num_devices=N_CORES)
    wst = nc.dram_tensor("wst", [2 * IN_SIZE, 2 * OUT_SIZE], BF16,
                         kind="ExternalInput").ap()
    biasd = nc.dram_tensor("biasd", [N_REL, OUT_SIZE], BF16,
                           kind="ExternalInput").ap()
    onesd = nc.dram_tensor("onesd", [N_REL, W], BF16,
                           kind="ExternalInput").ap()
    iotad = nc.dram_tensor("iotad", [P, GW * W], BF16,
                           kind="ExternalInput").ap()
    emsg = nc.dram_tensor("emsg", [P, T * IN_SIZE], BF16,
                          kind="ExternalInput").ap()
    ewlc = nc.dram_tensor("ewlc", [P, T], BF16, kind="ExternalInput").ap()
    eval_ = nc.dram_tensor("eval", [P, T], BF16, kind="ExternalInput").ap()
    n_wcol = (n_win + WPP - 1) // WPP
    out = nc.dram_tensor("out", [P, n_wcol * OUT_SIZE], F32,
                         kind="ExternalOutput").ap()

    ngroups = (T + GW - 1) // GW
    # metadata chunk size in groups / slots (chunks aligned to GW)
    cg = (ngroups + NMC - 1) // NMC
    mc = cg * GW

    with tile.TileContext(nc) as tc, ExitStack() as ctx:
        p_c = ctx.enter_context(tc.tile_pool(name="p_c", bufs=1))
        p_msg = ctx.enter_context(tc.tile_pool(name="p_msg", bufs=5))
        p_sel = ctx.enter_context(tc.tile_pool(name="p_sel", bufs=5))
        p_agg = ctx.enter_context(tc.tile_pool(name="p_agg", bufs=6))
        ps_agg = ctx.enter_context(tc.tile_pool(name="ps_agg", bufs=6,
                                                space="PSUM"))
        ps_out = ctx.enter_context(tc.tile_pool(name="ps_out", bufs=2,
                                                space="PSUM"))

        iot = p_c.tile([P, GW * W], BF16)
        nc.sync.dma_start(iot[:], iotad[:])
        # metadata resident in SBUF as NMC chunk tiles (per-tile deps let
        # early sel builds start before later chunks land)
        wlc_t, vlc_t = [], []
        for c in range(NMC):
            c0, c1 = c * mc, min((c + 1) * mc, T)
            wl_c = p_c.tile([P, mc], BF16, tag=f"wlc{c}")
            vl_c = p_c.tile([P, mc], BF16, tag=f"vlc{c}")
            wlc_t.append(wl_c)
            vlc_t.append(vl_c)
            if c > 0:
                continue
            nc.sync.dma_start(wl_c[:, :c1 - c0], ewlc[:, c0:c1])
            nc.sync.dma_start(vl_c[:, :c1 - c0], eval_[:, c0:c1])

        groups = {}

        def get_group(g):
            if g not in groups:
                g0, g1 = g * GW, min((g + 1) * GW, T)
                nb = g1 - g0
                mt = p_msg.tile([P, GW * IN_SIZE], BF16, tag="msg")
                nc.sync.dma_start(mt[:, :nb * IN_SIZE],
                                  emsg[:, g0 * IN_SIZE:g1 * IN_SIZE])
                # selection built j-major: st[p, j*nb + v]; DVE broadcast
                # operands keep an inner dense step (2x mode)
                st = p_sel.tile([P, GW * W], BF16, tag="sel")
                s3 = st[:, :nb * W].rearrange("p (j v) -> p j v", j=W)
                i3 = iot[:, :nb * W].rearrange("p (j v) -> p j v", j=W)
                ci = g // cg
                o0 = g0 - ci * mc
                bw = wlc_t[ci][:, o0:o0 + nb].unsqueeze(1).broadcast_to(
                    [P, W, nb])
                bv = vlc_t[ci][:, o0:o0 + nb].unsqueeze(1).broadcast_to(
                    [P, W, nb])
                nc.vector.tensor_tensor(s3, i3, bw,
                                        op=mybir.AluOpType.is_equal)
                nc.vector.tensor_tensor(s3, s3, bv, op=mybir.AluOpType.mult)
                groups[g] = (mt, st, nb)
            return groups[g]

        def cell_ap(b):
            mt, st, nb = get_group(b // GW)
            o = b % GW
            return (mt[:, o * IN_SIZE:(o + 1) * IN_SIZE],
                    st[:, :nb * W].rearrange("p (j v) -> p j v", j=W)[:, :, o])

        # prime the pipeline before emitting the remaining constant loads
        for g in range(min(4, ngroups)):
            get_group(g)

        wt = p_c.tile([2 * IN_SIZE, 2 * OUT_SIZE], BF16)
        nc.sync.dma_start(wt[:], wst[:])
        bt = p_c.tile([N_REL, OUT_SIZE], BF16)
        nc.sync.dma_start(bt[:], biasd[:])
        ot = p_c.tile([N_REL, W], BF16)
        nc.sync.dma_start(ot[:], onesd[:])
        for c in range(1, NMC):
            c0, c1 = c * mc, min((c + 1) * mc, T)
            nc.sync.dma_start(wlc_t[c][:, :c1 - c0], ewlc[:, c0:c1])
            nc.sync.dma_start(vlc_t[c][:, :c1 - c0], eval_[:, c0:c1])
        outsb = p_c.tile([P, n_wcol * OUT_SIZE], F32)

        # bias summed over relations once: pb = ones.T @ bias  [W, OUT]
        po = ps_out.tile([W, OUT_SIZE], F32)
        nc.tensor.matmul(out=po[:], lhsT=ot[:], rhs=bt[:],
                         start=True, stop=True)
        bb32 = p_c.tile([W, OUT_SIZE], F32)
        nc.vector.tensor_copy(bb32[:], po[:])
        # replicate to all 128 partitions (partition-offset copies)
        bbt = p_c.tile([P, OUT_SIZE], F32)
        k0 = 0
        while k0 < P:
            sz = min(W, P - k0)
            nc.vector.tensor_copy(bbt[k0:k0 + sz, :], bb32[:sz, :])
            k0 += sz

        for g in range(4, ngroups):
            get_group(g)

        for w in range(n_win):
            aggs = []
            for half in range(2):
                ps = ps_agg.tile([P, W], F32)
                for rr in range(2):
                    c2 = w * N_REL + 2 * half + rr
                    b0, b1 = int(starts[c2]), int(starts[c2 + 1])
                    for k, b in enumerate(range(b0, b1)):
                        m_ap, s_ap = cell_ap(b)
                        nc.tensor.matmul(
                            out=ps[rr * IN_SIZE:(rr + 1) * IN_SIZE, :],
                            lhsT=m_ap, rhs=s_ap,
                            tile_position=(0, rr * IN_SIZE),
                            start=(k == 0), stop=(b == b1 - 1))
                agg = p_agg.tile([P, W], BF16, tag="agg")
                nc.scalar.copy(agg[:], ps[:])
                aggs.append(agg)
            po = ps_out.tile([W, OUT_SIZE], F32)
            nc.tensor.matmul(out=po[:], lhsT=aggs[0][:],
                             rhs=wt[:, :OUT_SIZE], start=True, stop=False)
            nc.tensor.matmul(out=po[:], lhsT=aggs[1][:],
                             rhs=wt[:, OUT_SIZE:], start=False, stop=True)
            nc.scalar.copy(
                outsb[(w % WPP) * W:(w % WPP) * W + W,
                      (w // WPP) * OUT_SIZE:(w // WPP + 1) * OUT_SIZE],
                po[:])
        noc = 4
        oc = (n_wcol + noc - 1) // noc
        for c in range(noc):
            k0, k1 = c * oc, min((c + 1) * oc, n_wcol)
            c0, c1 = k0 * OUT_SIZE, k1 * OUT_SIZE
            o3 = outsb[:, c0:c1].rearrange("p (k o) -> p k o", o=OUT_SIZE)
            bb = bbt[:].unsqueeze(1).broadcast_to([P, k1 - k0, OUT_SIZE])
            nc.vector.tensor_tensor(o3, o3, bb, op=mybir.AluOpType.add)
            nc.sync.dma_start(out[:, c0:c1], outsb[:, c0:c1])

    nc.compile()
    _PROG_CACHE[key] = nc
    return nc


def _prepare(inp, src, dst, edge_val, weights, bias):
    inp = np.asarray(inp, dtype=np.float32)
    src = np.asarray(src)
    dst = np.asarray(dst)
    edge_val = np.asarray(edge_val, dtype=np.float32)
    weights = np.asarray(weights, dtype=np.float32)
    bias = np.asarray(bias, dtype=np.float32)

    n_win, starts, T, msg_all, wloc_all, val_all = _host_prep(
        inp, src, dst, edge_val)
    nc = _build_program(n_win, starts, T)

    bf16 = _bf16()
    wst = np.zeros((2 * IN_SIZE, 2 * OUT_SIZE), dtype=bf16)
    wst[:IN_SIZE, :OUT_SIZE] = weights[0].astype(bf16)
    wst[IN_SIZE:, :OUT_SIZE] = weights[1].astype(bf16)
    wst[:IN_SIZE, OUT_SIZE:] = weights[2].astype(bf16)
    wst[IN_SIZE:, OUT_SIZE:] = weights[3].astype(bf16)
    ones = np.ones((N_REL, W), dtype=bf16)
    # j-major iota: value at free position j*GW + v is j
    iota = np.broadcast_to(
        np.repeat(np.arange(W, dtype=np.float32), GW).astype(bf16),
        (P, GW * W)).copy()

    in_maps = []
    for c in range(N_CORES):
        in_maps.append({
            "wst": wst,
            "biasd": bias.astype(bf16),
            "onesd": ones,
            "iotad": iota,
            "emsg": msg_all[c].reshape(P, T * IN_SIZE),
            "ewlc": wloc_all[c],
            "eval": val_all[c],
        })
    return nc, in_maps, n_win


def _finish(res, n_win):
    n_wcol = (n_win + WPP - 1) // WPP
    parts = []
    for c in range(N_CORES):
        arr = res.results[c]["out"].reshape(P, n_wcol, OUT_SIZE)
        arr = arr[:WPP * W].reshape(WPP, W, n_wcol, OUT_SIZE)
        nodes = arr.transpose(2, 0, 1, 3).reshape(n_wcol * WPP * W, OUT_SIZE)
        parts.append(nodes[:NPC])
    return np.concatenate(parts, axis=0).astype(np.float32)


def kernel(inp, src, dst, edge_val, weights, bias):
    nc, in_maps, n_win = _prepare(inp, src, dst, edge_val, weights, bias)
    res = run_bass_kernel_spmd(nc, in_maps, list(range(N_CORES)))
    return _finish(res, n_win)
